# revision 6
# baseline (speedup 1.0000x reference)
"""Segment-mean (MeanToERA5) Trainium2 kernel.

Computes per-cluster means of a [32, 8, 512, 512] fp32 tensor over the
flattened 512x512 spatial axis, for 4096 clusters given by `mapping`
([262144] int), matching jax.ops.segment_sum(flat.T, mapping)/counts.

Strategy (8 NeuronCores, SPMD):
  - Host: stable-argsort `mapping`; group the 4096 clusters into groups of
    G=16 consecutive clusters; lay out the data cluster-sorted and
    transposed as rows of [256 batch] fp32, padded per-group to a uniform
    row count L = 128*CPG so the program structure is identical on every
    core. Each core owns 512 clusters = 32 groups.
  - Device: stream row-chunks of [128 rows, 256 batch]; for each chunk do
    two banded matmuls (batch halves) against a host-built [128, 16]
    weight matrix whose entries are 1/count(cluster) at the row's
    within-group column. PSUM accumulates the full [256, 512] result
    (2 banks); one copy + DMA out at the end.
"""

import sys
import time

if "/opt/trn_rl_repo" not in sys.path:
    sys.path.insert(0, "/opt/trn_rl_repo")

import numpy as np

import concourse.bacc as bacc
import concourse.tile as tile
from concourse import mybir
from concourse.bass_utils import run_bass_kernel_spmd

N_CLUSTERS = 4096
N = 512 * 512
B = 256
NCORES = 8
G = 16                      # clusters per group (= one-hot width)
GROUPS_PER_CORE = (N_CLUSTERS // NCORES) // G   # 32
CLUSTERS_PER_CORE = N_CLUSTERS // NCORES        # 512

_program_cache = {}
LAST_EXEC_NS = None


def _build_program(cpg: int):
    """Build the SPMD bass program for `cpg` 128-row chunks per group."""
    key = cpg
    if key in _program_cache:
        return _program_cache[key]

    L = 128 * cpg                      # rows per group (padded)
    R = GROUPS_PER_CORE * L            # rows per core

    nc = bacc.Bacc("TRN2", target_bir_lowering=False, debug=False,
                   num_devices=NCORES)
    x = nc.dram_tensor("x", [R, B], mybir.dt.float32, kind="ExternalInput")
    oh = nc.dram_tensor("oh", [R, G], mybir.dt.float32, kind="ExternalInput")
    out = nc.dram_tensor("out", [B, CLUSTERS_PER_CORE], mybir.dt.float32,
                         kind="ExternalOutput")

    # DRAM views: group-major -> [groups, 128 part, cpg, inner]
    xv = x.ap().rearrange("(g t p) b -> g p t b", t=cpg, p=128)
    ohv = oh.ap().rearrange("(g t p) w -> g p t w", t=cpg, p=128)
    outv = out.ap()

    last = GROUPS_PER_CORE * cpg - 1   # index of last chunk
    with tile.TileContext(nc) as tc:
        with (
            tc.tile_pool(name="xp", bufs=4) as xp,
            tc.tile_pool(name="ohp", bufs=4) as ohp,
            tc.tile_pool(name="ps", bufs=1, space="PSUM") as ps,
            tc.tile_pool(name="res", bufs=1) as resp,
        ):
            psum = [
                ps.tile([128, CLUSTERS_PER_CORE], mybir.dt.float32,
                        name=f"psum{h}", tag=f"psum{h}")
                for h in range(2)
            ]
            for g in range(GROUPS_PER_CORE):
                xt = xp.tile([128, cpg, B], mybir.dt.float32, tag="xt")
                nc.sync.dma_start(xt[:], xv[g])
                oht = ohp.tile([128, cpg, G], mybir.dt.float32, tag="oht")
                nc.sync.dma_start(oht[:], ohv[g])
                for t in range(cpg):
                    j = g * cpg + t
                    for h in range(2):
                        nc.tensor.matmul(
                            out=psum[h][:, g * G:(g + 1) * G],
                            lhsT=xt[:, t, h * 128:(h + 1) * 128],
                            rhs=oht[:, t, :],
                            start=(j == 0),
                            stop=(j == last),
                        )
            for h in range(2):
                res = resp.tile([128, CLUSTERS_PER_CORE], mybir.dt.float32,
                                name=f"res{h}", tag=f"res{h}")
                nc.vector.tensor_copy(res[:], psum[h][:])
                nc.sync.dma_start(outv[h * 128:(h + 1) * 128, :], res[:])

    nc.compile()
    _program_cache[key] = nc
    return nc


def _prepare(output: np.ndarray, mapping: np.ndarray):
    """Host prep: returns (nc, in_maps, cpg)."""
    t0 = time.time()
    assert output.shape == (32, 8, 512, 512) and output.dtype == np.float32
    mapping = np.asarray(mapping).astype(np.int64).ravel()
    assert mapping.shape == (N,)

    data2d = output.reshape(B, N)
    counts = np.bincount(mapping, minlength=N_CLUSTERS).astype(np.int64)
    assert counts.min() > 0, "empty cluster not supported"
    recip = (1.0 / counts).astype(np.float32)

    order = np.argsort(mapping, kind="stable")
    cum = np.zeros(N_CLUSTERS + 1, dtype=np.int64)
    np.cumsum(counts, out=cum[1:])

    # Group layout: 256 global groups of G=16 consecutive clusters.
    n_groups = N_CLUSTERS // G
    gstart = cum[::G]                      # [n_groups+1] row starts
    glen = np.diff(gstart)
    cpg = int(np.ceil(glen.max() / 128))
    L = 128 * cpg

    # Padded row-id table [n_groups, L]; -1 = padding.
    pad_rows = np.full((n_groups, L), -1, dtype=np.int64)
    col = np.arange(L)
    valid = col[None, :] < glen[:, None]
    flat_src = np.zeros((n_groups, L), dtype=np.int64)
    # rows of group g are order[gstart[g] : gstart[g]+glen[g]]
    flat_src[valid] = order[
        (gstart[:-1][:, None] + np.minimum(col[None, :], glen[:, None] - 1))[valid]
    ]
    pad_rows[valid] = flat_src[valid]
    pad_rows = pad_rows.reshape(-1)        # [n_groups * L]
    vmask = pad_rows >= 0

    # Gather data rows (transposed): x_all[r] = data2d[:, pad_rows[r]]
    dataT = np.ascontiguousarray(data2d.T)          # [N, B]
    x_all = np.zeros((n_groups * L, B), dtype=np.float32)
    x_all[vmask] = dataT[pad_rows[vmask]]

    # One-hot weights: 1/count at within-group column.
    clus = mapping[pad_rows[vmask]]
    oh_all = np.zeros((n_groups * L, G), dtype=np.float32)
    oh_all[np.nonzero(vmask)[0], clus % G] = recip[clus]

    t1 = time.time()
    nc = _build_program(cpg)
    t2 = time.time()

    rows_per_core = GROUPS_PER_CORE * L
    in_maps = []
    for k in range(NCORES):
        s = k * rows_per_core
        in_maps.append({
            "x": x_all[s:s + rows_per_core],
            "oh": oh_all[s:s + rows_per_core],
        })
    print(f"[kernel] host prep {t1 - t0:.2f}s  build+compile "
          f"{time.time() - t1:.2f}s  (cpg={cpg})", file=sys.stderr, flush=True)
    return nc, in_maps, cpg


def kernel(output: np.ndarray, mapping: np.ndarray) -> np.ndarray:
    nc, in_maps, _ = _prepare(output, mapping)
    t2 = time.time()
    res = run_bass_kernel_spmd(nc, in_maps, list(range(NCORES)))
    t3 = time.time()
    full = np.concatenate([res.results[k]["out"] for k in range(NCORES)],
                          axis=1)                   # [B, 4096]
    out = np.ascontiguousarray(full.reshape(32, 8, N_CLUSTERS))
    print(f"[kernel] run {t3 - t2:.2f}s", file=sys.stderr, flush=True)
    return out


# revision 8
# speedup vs baseline: 2.2141x; 2.2141x over previous
"""Segment-mean (MeanToERA5) Trainium2 kernel.

Computes per-cluster means of a [32, 8, 512, 512] fp32 tensor over the
flattened 512x512 spatial axis, for 4096 clusters given by `mapping`
([262144] int), matching jax.ops.segment_sum(flat.T, mapping)/counts.

Strategy (8 NeuronCores, SPMD):
  - Host: stable-argsort `mapping`; group the 4096 clusters into groups of
    G=16 consecutive clusters; lay out the data cluster-sorted and
    transposed as rows of [256 batch] fp32, padded per-group to a uniform
    row count L = 128*CPG so the program structure is identical on every
    core. Each core owns 512 clusters = 32 groups.
  - Device: stream row-chunks of [128 rows, 256 batch]; for each chunk do
    two banded matmuls (batch halves) against a host-built [128, 16]
    weight matrix whose entries are 1/count(cluster) at the row's
    within-group column. PSUM accumulates the full [256, 512] result
    (2 banks); one copy + DMA out at the end.
"""

import sys
import time

if "/opt/trn_rl_repo" not in sys.path:
    sys.path.insert(0, "/opt/trn_rl_repo")

import numpy as np

import concourse.bacc as bacc
import concourse.tile as tile
from concourse import mybir
from concourse.bass_utils import run_bass_kernel_spmd

N_CLUSTERS = 4096
N = 512 * 512
B = 256
NCORES = 8
G = 16                      # clusters per group (= one-hot width)
GROUPS_PER_CORE = (N_CLUSTERS // NCORES) // G   # 32
CLUSTERS_PER_CORE = N_CLUSTERS // NCORES        # 512

_program_cache = {}
LAST_EXEC_NS = None


def _build_program(cpg: int, loop: int = 1):
    """Build the SPMD bass program for `cpg` 128-row chunks per group.

    loop > 1 repeats the whole pipeline on-device (for benchmarking: one
    dispatch, `loop` executions)."""
    key = (cpg, loop)
    if key in _program_cache:
        return _program_cache[key]

    L = 128 * cpg                      # rows per group (padded)
    R = GROUPS_PER_CORE * L            # rows per core

    nc = bacc.Bacc("TRN2", target_bir_lowering=False, debug=False,
                   num_devices=NCORES)
    x = nc.dram_tensor("x", [R, B], mybir.dt.float32, kind="ExternalInput")
    oh = nc.dram_tensor("oh", [R, G], mybir.dt.float32, kind="ExternalInput")
    out = nc.dram_tensor("out", [B, CLUSTERS_PER_CORE], mybir.dt.float32,
                         kind="ExternalOutput")

    # DRAM views: group-major -> [groups, 128 part, cpg, inner]
    xv = x.ap().rearrange("(g t p) b -> g p t b", t=cpg, p=128)
    ohv = oh.ap().rearrange("(g t p) w -> g p t w", t=cpg, p=128)
    outv = out.ap()

    last = GROUPS_PER_CORE * cpg - 1   # index of last chunk
    with tile.TileContext(nc) as tc:
        with (
            tc.tile_pool(name="xp", bufs=4) as xp,
            tc.tile_pool(name="ohp", bufs=4) as ohp,
            tc.tile_pool(name="ps", bufs=1, space="PSUM") as ps,
            tc.tile_pool(name="res", bufs=1) as resp,
        ):
            def body(_i=None):
                psum = [
                    ps.tile([128, CLUSTERS_PER_CORE], mybir.dt.float32,
                            name=f"psum{h}", tag=f"psum{h}")
                    for h in range(2)
                ]
                for g in range(GROUPS_PER_CORE):
                    xt = xp.tile([128, cpg, B], mybir.dt.float32, tag="xt")
                    nc.sync.dma_start(xt[:], xv[g])
                    oht = ohp.tile([128, cpg, G], mybir.dt.float32, tag="oht")
                    nc.sync.dma_start(oht[:], ohv[g])
                    for t in range(cpg):
                        j = g * cpg + t
                        for h in range(2):
                            nc.tensor.matmul(
                                out=psum[h][:, g * G:(g + 1) * G],
                                lhsT=xt[:, t, h * 128:(h + 1) * 128],
                                rhs=oht[:, t, :],
                                start=(j == 0),
                                stop=(j == last),
                            )
                for h in range(2):
                    res = resp.tile([128, CLUSTERS_PER_CORE],
                                    mybir.dt.float32,
                                    name=f"res{h}", tag=f"res{h}")
                    nc.vector.tensor_copy(res[:], psum[h][:])
                    nc.sync.dma_start(outv[h * 128:(h + 1) * 128, :], res[:])

            if loop == 1:
                body()
            else:
                with tc.For_i(0, loop, 1) as i:
                    body(i)

    nc.compile()
    _program_cache[key] = nc
    return nc


def _prepare(output: np.ndarray, mapping: np.ndarray):
    """Host prep: returns (nc, in_maps, cpg)."""
    t0 = time.time()
    assert output.shape == (32, 8, 512, 512) and output.dtype == np.float32
    mapping = np.asarray(mapping).astype(np.int64).ravel()
    assert mapping.shape == (N,)

    data2d = output.reshape(B, N)
    counts = np.bincount(mapping, minlength=N_CLUSTERS).astype(np.int64)
    assert counts.min() > 0, "empty cluster not supported"
    recip = (1.0 / counts).astype(np.float32)

    order = np.argsort(mapping, kind="stable")
    cum = np.zeros(N_CLUSTERS + 1, dtype=np.int64)
    np.cumsum(counts, out=cum[1:])

    # Group layout: 256 global groups of G=16 consecutive clusters.
    n_groups = N_CLUSTERS // G
    gstart = cum[::G]                      # [n_groups+1] row starts
    glen = np.diff(gstart)
    cpg = int(np.ceil(glen.max() / 128))
    L = 128 * cpg

    # Padded row-id table [n_groups, L]; -1 = padding.
    pad_rows = np.full((n_groups, L), -1, dtype=np.int64)
    col = np.arange(L)
    valid = col[None, :] < glen[:, None]
    flat_src = np.zeros((n_groups, L), dtype=np.int64)
    # rows of group g are order[gstart[g] : gstart[g]+glen[g]]
    flat_src[valid] = order[
        (gstart[:-1][:, None] + np.minimum(col[None, :], glen[:, None] - 1))[valid]
    ]
    pad_rows[valid] = flat_src[valid]
    pad_rows = pad_rows.reshape(-1)        # [n_groups * L]
    vmask = pad_rows >= 0

    # Gather data rows (transposed): x_all[r] = data2d[:, pad_rows[r]]
    dataT = np.ascontiguousarray(data2d.T)          # [N, B]
    x_all = np.zeros((n_groups * L, B), dtype=np.float32)
    x_all[vmask] = dataT[pad_rows[vmask]]

    # One-hot weights: 1/count at within-group column.
    clus = mapping[pad_rows[vmask]]
    oh_all = np.zeros((n_groups * L, G), dtype=np.float32)
    oh_all[np.nonzero(vmask)[0], clus % G] = recip[clus]

    t1 = time.time()
    nc = _build_program(cpg)
    t2 = time.time()

    rows_per_core = GROUPS_PER_CORE * L
    in_maps = []
    for k in range(NCORES):
        s = k * rows_per_core
        in_maps.append({
            "x": x_all[s:s + rows_per_core],
            "oh": oh_all[s:s + rows_per_core],
        })
    print(f"[kernel] host prep {t1 - t0:.2f}s  build+compile "
          f"{time.time() - t1:.2f}s  (cpg={cpg})", file=sys.stderr, flush=True)
    return nc, in_maps, cpg


def kernel(output: np.ndarray, mapping: np.ndarray) -> np.ndarray:
    nc, in_maps, _ = _prepare(output, mapping)
    t2 = time.time()
    res = run_bass_kernel_spmd(nc, in_maps, list(range(NCORES)))
    t3 = time.time()
    full = np.concatenate([res.results[k]["out"] for k in range(NCORES)],
                          axis=1)                   # [B, 4096]
    out = np.ascontiguousarray(full.reshape(32, 8, N_CLUSTERS))
    print(f"[kernel] run {t3 - t2:.2f}s", file=sys.stderr, flush=True)
    return out


# revision 11
# speedup vs baseline: 3.7368x; 1.6878x over previous
"""Segment-mean (MeanToERA5) Trainium2 kernel.

Computes per-cluster means of a [32, 8, 512, 512] fp32 tensor over the
flattened 512x512 spatial axis, for 4096 clusters given by `mapping`
([262144] int), matching jax.ops.segment_sum(flat.T, mapping)/counts.

Strategy (8 NeuronCores, SPMD):
  - Host: stable-argsort `mapping`; group the 4096 clusters into groups of
    G=16 consecutive clusters; lay out the data cluster-sorted and
    transposed as rows of [256 batch] fp32, padded per-group to a uniform
    row count L = 128*cpg so the program structure is identical on every
    core. Each core owns 512 clusters = 32 groups. Both device inputs are
    packed partition-major on the host so all DMAs are fully contiguous.
  - Device: per 128-row chunk, one fp32 matmul: stationary = [128, 16]
    one-hot (value 1/count at the row's within-group column), moving =
    data chunk [128, 256]. PSUM accumulates [512 clusters, 256 batch]
    c-major in 4 [128, 256] tiles (2 banks); copy + DMA out at the end.
  - Host: assemble [4096, 256], transpose to [256, 4096] (the unshard).
"""

import sys
import time

if "/opt/trn_rl_repo" not in sys.path:
    sys.path.insert(0, "/opt/trn_rl_repo")

import numpy as np

import concourse.bacc as bacc
import concourse.tile as tile
from concourse import mybir
from concourse.bass_utils import run_bass_kernel_spmd

N_CLUSTERS = 4096
N = 512 * 512
B = 256
NCORES = 8
G = 32                      # clusters per group (= one-hot width)
GROUPS_PER_CORE = (N_CLUSTERS // NCORES) // G   # 32
CLUSTERS_PER_CORE = N_CLUSTERS // NCORES        # 512
NQ = CLUSTERS_PER_CORE // 128                   # psum tiles (4)

_program_cache = {}
LAST_EXEC_NS = None


def _build_program(cpg: int, loop: int = 1):
    """Build the SPMD bass program for `cpg` 128-row chunks per group.

    loop > 1 repeats the whole pipeline on-device (for benchmarking: one
    dispatch, `loop` executions)."""
    key = (cpg, loop)
    if key in _program_cache:
        return _program_cache[key]

    L = 128 * cpg                      # rows per group (padded)
    nchunks = GROUPS_PER_CORE * cpg    # chunks per core
    gpq = 128 // G                     # groups per psum tile (8)

    nc = bacc.Bacc("TRN2", target_bir_lowering=False, debug=False,
                   num_devices=NCORES)
    # x packed as [groups, 128 partitions, cpg*B] (host pre-permuted)
    x = nc.dram_tensor("x", [GROUPS_PER_CORE, 128, cpg * B],
                       mybir.dt.float32, kind="ExternalInput")
    # oh packed as [128 partitions, nchunks*G]
    oh = nc.dram_tensor("oh", [128, nchunks * G], mybir.dt.float32,
                        kind="ExternalInput")
    # output c-major: [512 clusters, 256 batch]
    out = nc.dram_tensor("out", [CLUSTERS_PER_CORE, B], mybir.dt.float32,
                         kind="ExternalOutput")

    xv = x.ap()
    ohv = oh.ap()
    outv = out.ap()

    with tile.TileContext(nc) as tc:
        with (
            tc.tile_pool(name="xp", bufs=4) as xp,
            tc.tile_pool(name="ohp", bufs=1) as ohp,
            tc.tile_pool(name="ps", bufs=1, space="PSUM") as ps,
            tc.tile_pool(name="res", bufs=2) as resp,
        ):
            def body(_i=None):
                oht = ohp.tile([128, nchunks * G], mybir.dt.float32,
                               name="oht", tag="oht")
                nc.sync.dma_start(oht[:], ohv[:])
                psum = [
                    ps.tile([128, B], mybir.dt.float32,
                            name=f"psum{q}", tag=f"psum{q}")
                    for q in range(NQ)
                ]
                for g in range(GROUPS_PER_CORE):
                    q, gq = divmod(g, gpq)
                    po = gq * G        # partition offset within psum tile
                    xt = xp.tile([128, cpg * B], mybir.dt.float32, tag="xt")
                    nc.sync.dma_start(xt[:], xv[g])
                    for t in range(cpg):
                        j = g * cpg + t
                        nc.tensor.matmul(
                            out=psum[q][po:po + G, :],
                            lhsT=oht[:, j * G:(j + 1) * G],
                            rhs=xt[:, t * B:(t + 1) * B],
                            start=(t == 0),
                            stop=(t == cpg - 1),
                            tile_position=(0, po),
                        )
                for q in range(NQ):
                    res = resp.tile([128, B], mybir.dt.float32,
                                    name=f"res{q}", tag="res")
                    nc.vector.tensor_copy(res[:], psum[q][:])
                    nc.sync.dma_start(outv[q * 128:(q + 1) * 128, :], res[:])

            if loop == 1:
                body()
            else:
                with tc.For_i(0, loop, 1) as i:
                    body(i)

    nc.compile()
    _program_cache[key] = nc
    return nc


def _prepare(output: np.ndarray, mapping: np.ndarray):
    """Host prep: returns (nc, in_maps, cpg)."""
    t0 = time.time()
    assert output.shape == (32, 8, 512, 512) and output.dtype == np.float32
    mapping = np.asarray(mapping).astype(np.int64).ravel()
    assert mapping.shape == (N,)

    data2d = output.reshape(B, N)
    counts = np.bincount(mapping, minlength=N_CLUSTERS).astype(np.int64)
    assert counts.min() > 0, "empty cluster not supported"
    recip = (1.0 / counts).astype(np.float32)

    order = np.argsort(mapping, kind="stable")
    cum = np.zeros(N_CLUSTERS + 1, dtype=np.int64)
    np.cumsum(counts, out=cum[1:])

    # Group layout: global groups of G consecutive clusters.
    n_groups = N_CLUSTERS // G
    gstart = cum[::G]                      # [n_groups+1] row starts
    glen = np.diff(gstart)
    cpg = int(np.ceil(glen.max() / 128))
    L = 128 * cpg

    # Padded row-id table [n_groups, L]; -1 = padding.
    pad_rows = np.full((n_groups, L), -1, dtype=np.int64)
    col = np.arange(L)
    valid = col[None, :] < glen[:, None]
    flat_src = np.zeros((n_groups, L), dtype=np.int64)
    flat_src[valid] = order[
        (gstart[:-1][:, None] + np.minimum(col[None, :], glen[:, None] - 1))[valid]
    ]
    pad_rows[valid] = flat_src[valid]
    pad_rows = pad_rows.reshape(-1)        # [n_groups * L]
    vmask = pad_rows >= 0

    # Gather data rows (transposed): x_all[r] = data2d[:, pad_rows[r]]
    dataT = np.ascontiguousarray(data2d.T)          # [N, B]
    x_all = np.zeros((n_groups * L, B), dtype=np.float32)
    x_all[vmask] = dataT[pad_rows[vmask]]
    # pack partition-major: [g, t, p, b] -> [g, p, t*B + b]
    x_all = np.ascontiguousarray(
        x_all.reshape(n_groups, cpg, 128, B).transpose(0, 2, 1, 3)
    ).reshape(n_groups, 128, cpg * B)

    # One-hot weights: 1/count at within-group column.
    clus = mapping[pad_rows[vmask]]
    oh_all = np.zeros((n_groups * L, G), dtype=np.float32)
    oh_all[np.nonzero(vmask)[0], clus % G] = recip[clus]
    # pack per core: [gc, t, p, w] -> [p, (gc*cpg + t)*G + w]
    nchunks = GROUPS_PER_CORE * cpg
    oh_all = np.ascontiguousarray(
        oh_all.reshape(NCORES, GROUPS_PER_CORE * cpg, 128, G)
        .transpose(0, 2, 1, 3)
    ).reshape(NCORES, 128, nchunks * G)

    t1 = time.time()
    nc = _build_program(cpg)

    in_maps = []
    for k in range(NCORES):
        in_maps.append({
            "x": x_all[k * GROUPS_PER_CORE:(k + 1) * GROUPS_PER_CORE],
            "oh": oh_all[k],
        })
    print(f"[kernel] host prep {t1 - t0:.2f}s  build+compile "
          f"{time.time() - t1:.2f}s  (cpg={cpg})", file=sys.stderr, flush=True)
    return nc, in_maps, cpg


def kernel(output: np.ndarray, mapping: np.ndarray) -> np.ndarray:
    nc, in_maps, _ = _prepare(output, mapping)
    t2 = time.time()
    res = run_bass_kernel_spmd(nc, in_maps, list(range(NCORES)))
    t3 = time.time()
    full = np.concatenate([res.results[k]["out"] for k in range(NCORES)],
                          axis=0)                   # [4096, 256] c-major
    out = np.ascontiguousarray(full.T).reshape(32, 8, N_CLUSTERS)
    print(f"[kernel] run {t3 - t2:.2f}s", file=sys.stderr, flush=True)
    return out


# revision 12
# speedup vs baseline: 4.8369x; 1.2944x over previous
"""Segment-mean (MeanToERA5) Trainium2 kernel.

Computes per-cluster means of a [32, 8, 512, 512] fp32 tensor over the
flattened 512x512 spatial axis, for 4096 clusters given by `mapping`
([262144] int), matching jax.ops.segment_sum(flat.T, mapping)/counts.

Strategy (8 NeuronCores, SPMD):
  - Host: stable-argsort `mapping`; group the 4096 clusters into groups of
    G=32 consecutive clusters; lay out the data cluster-sorted and
    transposed as rows of [256 batch] fp32, padded per-group to a uniform
    row count 128*cpg so the program structure is identical on every
    core. Each core owns 512 clusters = 16 groups. Inputs are packed
    partition-major on the host so all DMAs are fully contiguous.
  - Device: build the per-chunk [128, 32] one-hot weights on DVE from
    compact (column-id, 1/count) vectors; per 128-row chunk one fp32
    matmul: stationary = one-hot, moving = data chunk [128, 256]. PSUM
    accumulates [512 clusters, 256 batch] c-major in 4 [128, 256] tiles;
    copy + DMA out at the end.
  - Host: assemble [4096, 256], transpose to [256, 4096] (the unshard).
"""

import sys
import time

if "/opt/trn_rl_repo" not in sys.path:
    sys.path.insert(0, "/opt/trn_rl_repo")

import numpy as np

import concourse.bacc as bacc
import concourse.tile as tile
from concourse import mybir
from concourse.bass_utils import run_bass_kernel_spmd

N_CLUSTERS = 4096
N = 512 * 512
B = 256
NCORES = 8
G = 32                      # clusters per group (= one-hot width)
GROUPS_PER_CORE = (N_CLUSTERS // NCORES) // G   # 16
CLUSTERS_PER_CORE = N_CLUSTERS // NCORES        # 512
NQ = CLUSTERS_PER_CORE // 128                   # psum tiles (4)

_program_cache = {}
LAST_EXEC_NS = None


def _build_program(cpg: int, loop: int = 1):
    """Build the SPMD bass program for `cpg` 128-row chunks per group.

    loop > 1 repeats the whole pipeline on-device (for benchmarking: one
    dispatch, `loop` executions)."""
    key = (cpg, loop)
    if key in _program_cache:
        return _program_cache[key]

    nchunks = GROUPS_PER_CORE * cpg    # chunks per core
    gpq = 128 // G                     # groups per psum tile (4)

    nc = bacc.Bacc("TRN2", target_bir_lowering=False, debug=False,
                   num_devices=NCORES)
    # x packed as [groups, 128 partitions, cpg*B] (host pre-permuted)
    x = nc.dram_tensor("x", [GROUPS_PER_CORE, 128, cpg * B],
                       mybir.dt.float32, kind="ExternalInput")
    # per-row one-hot column id and value, packed [128, nchunks]
    cid = nc.dram_tensor("cid", [128, nchunks], mybir.dt.float32,
                         kind="ExternalInput")
    val = nc.dram_tensor("val", [128, nchunks], mybir.dt.float32,
                         kind="ExternalInput")
    iota = nc.dram_tensor("iota", [128, G], mybir.dt.float32,
                          kind="ExternalInput")
    # output c-major: [512 clusters, 256 batch]
    out = nc.dram_tensor("out", [CLUSTERS_PER_CORE, B], mybir.dt.float32,
                         kind="ExternalOutput")

    xv, outv = x.ap(), out.ap()

    with tile.TileContext(nc) as tc:
        with (
            tc.tile_pool(name="xp", bufs=4) as xp,
            tc.tile_pool(name="ohp", bufs=1) as ohp,
            tc.tile_pool(name="ps", bufs=1, space="PSUM") as ps,
            tc.tile_pool(name="res", bufs=2) as resp,
        ):
            def body(_i=None):
                cidt = ohp.tile([128, nchunks], mybir.dt.float32,
                                name="cidt", tag="cidt")
                nc.sync.dma_start(cidt[:], cid.ap())
                valt = ohp.tile([128, nchunks], mybir.dt.float32,
                                name="valt", tag="valt")
                nc.sync.dma_start(valt[:], val.ap())
                iot = ohp.tile([128, G], mybir.dt.float32,
                               name="iot", tag="iot")
                nc.sync.dma_start(iot[:], iota.ap())
                # expand to one-hot weights [128, nchunks, G]
                ohx = ohp.tile([128, nchunks, G], mybir.dt.float32,
                               name="ohx", tag="ohx")
                nc.vector.tensor_tensor(
                    out=ohx[:],
                    in0=cidt[:].unsqueeze(2).broadcast_to([128, nchunks, G]),
                    in1=iot[:].unsqueeze(1).broadcast_to([128, nchunks, G]),
                    op=mybir.AluOpType.is_equal,
                )
                nc.vector.tensor_tensor(
                    out=ohx[:],
                    in0=ohx[:],
                    in1=valt[:].unsqueeze(2).broadcast_to([128, nchunks, G]),
                    op=mybir.AluOpType.mult,
                )
                psum = [
                    ps.tile([128, B], mybir.dt.float32,
                            name=f"psum{q}", tag=f"psum{q}")
                    for q in range(NQ)
                ]
                for g in range(GROUPS_PER_CORE):
                    q, gq = divmod(g, gpq)
                    po = gq * G        # partition offset within psum tile
                    xt = xp.tile([128, cpg * B], mybir.dt.float32, tag="xt")
                    nc.sync.dma_start(xt[:], xv[g])
                    for t in range(cpg):
                        j = g * cpg + t
                        nc.tensor.matmul(
                            out=psum[q][po:po + G, :],
                            lhsT=ohx[:, j, :],
                            rhs=xt[:, t * B:(t + 1) * B],
                            start=(t == 0),
                            stop=(t == cpg - 1),
                            tile_position=(0, po),
                        )
                for q in range(NQ):
                    res = resp.tile([128, B], mybir.dt.float32,
                                    name=f"res{q}", tag="res")
                    nc.vector.tensor_copy(res[:], psum[q][:])
                    nc.sync.dma_start(outv[q * 128:(q + 1) * 128, :], res[:])

            if loop == 1:
                body()
            else:
                with tc.For_i(0, loop, 1) as i:
                    body(i)

    nc.compile()
    _program_cache[key] = nc
    return nc


def _prepare(output: np.ndarray, mapping: np.ndarray):
    """Host prep: returns (nc, in_maps, cpg)."""
    t0 = time.time()
    assert output.shape == (32, 8, 512, 512) and output.dtype == np.float32
    mapping = np.asarray(mapping).astype(np.int64).ravel()
    assert mapping.shape == (N,)

    data2d = output.reshape(B, N)
    counts = np.bincount(mapping, minlength=N_CLUSTERS).astype(np.int64)
    assert counts.min() > 0, "empty cluster not supported"
    recip = (1.0 / counts).astype(np.float32)

    order = np.argsort(mapping, kind="stable")
    cum = np.zeros(N_CLUSTERS + 1, dtype=np.int64)
    np.cumsum(counts, out=cum[1:])

    # Group layout: global groups of G consecutive clusters.
    n_groups = N_CLUSTERS // G
    gstart = cum[::G]                      # [n_groups+1] row starts
    glen = np.diff(gstart)
    cpg = int(np.ceil(glen.max() / 128))
    L = 128 * cpg

    # Padded row-id table [n_groups, L]; -1 = padding.
    pad_rows = np.full((n_groups, L), -1, dtype=np.int64)
    col = np.arange(L)
    valid = col[None, :] < glen[:, None]
    flat_src = np.zeros((n_groups, L), dtype=np.int64)
    flat_src[valid] = order[
        (gstart[:-1][:, None] + np.minimum(col[None, :], glen[:, None] - 1))[valid]
    ]
    pad_rows[valid] = flat_src[valid]
    pad_rows = pad_rows.reshape(-1)        # [n_groups * L]
    vmask = pad_rows >= 0

    # Gather data rows (transposed): x_all[r] = data2d[:, pad_rows[r]]
    dataT = np.ascontiguousarray(data2d.T)          # [N, B]
    x_all = np.zeros((n_groups * L, B), dtype=np.float32)
    x_all[vmask] = dataT[pad_rows[vmask]]
    # pack partition-major: [g, t, p, b] -> [g, p, t*B + b]
    x_all = np.ascontiguousarray(
        x_all.reshape(n_groups, cpg, 128, B).transpose(0, 2, 1, 3)
    ).reshape(n_groups, 128, cpg * B)

    # Compact one-hot: per-row within-group column id and value 1/count.
    cid_all = np.zeros(n_groups * L, dtype=np.float32)
    val_all = np.zeros(n_groups * L, dtype=np.float32)
    clus = mapping[pad_rows[vmask]]
    cid_all[vmask] = (clus % G).astype(np.float32)
    val_all[vmask] = recip[clus]
    # pack [rows] -> [core][p][chunk]
    nchunks = GROUPS_PER_CORE * cpg

    def pack(a):
        return np.ascontiguousarray(
            a.reshape(NCORES, nchunks, 128).transpose(0, 2, 1))

    cid_all = pack(cid_all)
    val_all = pack(val_all)
    iota_np = np.broadcast_to(np.arange(G, dtype=np.float32), (128, G)).copy()

    t1 = time.time()
    nc = _build_program(cpg)

    in_maps = []
    for k in range(NCORES):
        in_maps.append({
            "x": x_all[k * GROUPS_PER_CORE:(k + 1) * GROUPS_PER_CORE],
            "cid": cid_all[k],
            "val": val_all[k],
            "iota": iota_np,
        })
    print(f"[kernel] host prep {t1 - t0:.2f}s  build+compile "
          f"{time.time() - t1:.2f}s  (cpg={cpg})", file=sys.stderr, flush=True)
    return nc, in_maps, cpg


def kernel(output: np.ndarray, mapping: np.ndarray) -> np.ndarray:
    nc, in_maps, _ = _prepare(output, mapping)
    t2 = time.time()
    res = run_bass_kernel_spmd(nc, in_maps, list(range(NCORES)))
    t3 = time.time()
    full = np.concatenate([res.results[k]["out"] for k in range(NCORES)],
                          axis=0)                   # [4096, 256] c-major
    out = np.ascontiguousarray(full.T).reshape(32, 8, N_CLUSTERS)
    print(f"[kernel] run {t3 - t2:.2f}s", file=sys.stderr, flush=True)
    return out
